# revision 1
# baseline (speedup 1.0000x reference)
"""Trainium2 Bass kernel for sharded multi-head attention (8 NeuronCores).

Contract: kernel(**inputs) takes the FULL inputs (Q,K,V,mask,W*,b*) and
returns the full (x, dist) outputs, matching reference.reference().

Sharding: 8 cores; core c -> batch b = c//2, head-group g = c%2 (4 heads
per core). Each core computes its 4 heads' dist rows and an x partial;
the host sums the two x partials per batch and adds bo.
"""
"""Bass/Tile kernel builder for sharded multi-head attention on TRN2.

Sharding (8 cores): core c -> batch b = c//2, head-group g = c%2 (4 heads).
Each core computes, for its 4 heads:
  qT/kT [hd, s] and v [s, hd] projections,
  phase A (q-major): masked scores -> exp (+row sums) -> dist -> DRAM,
  phase B (k-major): masked scoresT -> exp -> attnT = v.T @ eT (unnormalized),
  phase C: x_partial = sum_h (recip_h * attnT_h).T @ Wo_h.T -> DRAM.
Host sums the two x partials per batch and adds bo.

Layout conventions:
  qt_sb/kt_sb [128p, NG, S]: partition = dout within 128-group, head h lives at
    group h//2, partitions 64*(h%2) .. +64.
  v_sb [128p, NQB, NHC, HD]: partition = k within 128-block.
  atn [64p, NHC, S]: partition = hd, per-head attnT scaled by 1/rowsum.
Masking is folded into the PE: scores_psum += (-8e10*I) @ mask_tile (bf16),
so exp(psum/8) yields exactly 0 for masked entries.
"""

from contextlib import ExitStack

import numpy as np

import concourse.bass as bass
import concourse.tile as tile
from concourse import bacc, mybir

F32 = mybir.dt.float32
F32R = mybir.dt.float32r
BF16 = mybir.dt.bfloat16
U8 = mybir.dt.uint8
AF = mybir.ActivationFunctionType
ALU = mybir.AluOpType

NEG_BIG = -8.0e10  # pre-scale mask additive constant; /8 -> -1e10


def build_nc(S=2048, DM=512, HD=64, NHC=4, num_devices=8):
    """Build the per-core Bass program. Returns the compiled Bacc object."""
    HG = NHC * HD            # head dims per core (256)
    NG = HG // 128           # dout groups of 128 (2)
    DC = DM // 128           # din chunks (4)
    NQB = S // 128           # 128-row blocks (16)
    NSC = S // 512           # 512-col chunks (4)
    PB = 1024 if S % 1024 == 0 else 512   # psum block width
    NPB = S // PB            # psum blocks per row
    NCH = PB // 512          # 512-chunks per psum block

    nc = bacc.Bacc("TRN2", target_bir_lowering=False, debug=False,
                   num_devices=num_devices)

    qt_in = nc.dram_tensor("qt_in", [DM, S], F32, kind="ExternalInput")
    kt_in = nc.dram_tensor("kt_in", [DM, S], F32, kind="ExternalInput")
    vt_in = nc.dram_tensor("vt_in", [DM, S], F32, kind="ExternalInput")
    maskq_in = nc.dram_tensor("maskq_in", [S, S], U8, kind="ExternalInput")
    maskt_in = nc.dram_tensor("maskt_in", [S, S], U8, kind="ExternalInput")
    wqt_in = nc.dram_tensor("wqt_in", [DM, HG], F32, kind="ExternalInput")
    wkt_in = nc.dram_tensor("wkt_in", [DM, HG], F32, kind="ExternalInput")
    wvt_in = nc.dram_tensor("wvt_in", [DM, HG], F32, kind="ExternalInput")
    wot_in = nc.dram_tensor("wot_in", [HG, DM], F32, kind="ExternalInput")
    bq_in = nc.dram_tensor("bq_in", [128, NG], F32, kind="ExternalInput")
    bk_in = nc.dram_tensor("bk_in", [128, NG], F32, kind="ExternalInput")
    bv_in = nc.dram_tensor("bv_in", [1, HG], F32, kind="ExternalInput")
    negi_in = nc.dram_tensor("negi_in", [128, 128], BF16, kind="ExternalInput")

    dist_out = nc.dram_tensor("dist_out", [NHC, S, S], F32, kind="ExternalOutput")
    x_out = nc.dram_tensor("x_out", [S, DM], F32, kind="ExternalOutput")
    # round-trip scratch for turning per-qb row-sum recips [128,16] into a
    # DRAM row [S] that can be broadcast-read across partitions
    recip_dram = nc.dram_tensor("recip_scratch", [NHC, S], F32)

    def r(ap):
        return ap.bitcast(F32R)

    with tile.TileContext(nc) as tc, ExitStack() as ctx:
        persist = ctx.enter_context(tc.tile_pool(name="persist", bufs=1))
        psp = ctx.enter_context(tc.tile_pool(name="psp", bufs=2, space="PSUM"))
        pap = ctx.enter_context(tc.tile_pool(name="pap", bufs=1, space="PSUM"))

        negI = persist.tile([128, 128], BF16)
        nc.sync.dma_start(out=negI[:], in_=negi_in.ap())

        wq_sb = persist.tile([128, DC, HG], F32)
        wk_sb = persist.tile([128, DC, HG], F32)
        wv_sb = persist.tile([128, DC, HG], F32)
        nc.sync.dma_start(out=wq_sb[:], in_=wqt_in.ap().rearrange("(c p) m -> p c m", p=128))
        nc.sync.dma_start(out=wk_sb[:], in_=wkt_in.ap().rearrange("(c p) m -> p c m", p=128))
        nc.sync.dma_start(out=wv_sb[:], in_=wvt_in.ap().rearrange("(c p) m -> p c m", p=128))
        wo_sb = persist.tile([64, NHC, DM], F32)
        nc.sync.dma_start(out=wo_sb[:], in_=wot_in.ap().rearrange("(h p) m -> p h m", p=64))
        bq_sb = persist.tile([128, NG], F32)
        bk_sb = persist.tile([128, NG], F32)
        nc.sync.dma_start(out=bq_sb[:], in_=bq_in.ap())
        nc.sync.dma_start(out=bk_sb[:], in_=bk_in.ap())
        bv_sb = persist.tile([128, HG], F32)
        bv_bcast = bass.AP(tensor=bv_in.ap().tensor, offset=0,
                           ap=[[0, 128], [1, HG]])
        nc.gpsimd.dma_start(out=bv_sb[:], in_=bv_bcast)

        qt_sb = persist.tile([128, NG, S], F32)
        kt_sb = persist.tile([128, NG, S], F32)
        v_sb = persist.tile([128, NQB, NHC, HD], F32)
        atn = persist.tile([64, NHC, S], F32)
        recip_all = persist.tile([128, NHC, NQB], F32)

        # ---------------- projections ----------------
        with tc.tile_pool(name="proj", bufs=3) as projp:
            # q/k: out[dout, s] = W[dout, :] @ X[:, s]  (contract din chunks)
            for tname, x_in, w_sb, b_sb, o_sb in (
                ("q", qt_in, wq_sb, bq_sb, qt_sb),
                ("k", kt_in, wk_sb, bk_sb, kt_sb),
            ):
                for sc in range(NSC):
                    xc = projp.tile([128, DC, 512], F32, tag="xc")
                    nc.sync.dma_start(
                        out=xc[:],
                        in_=x_in.ap().rearrange("(c p) s -> p c s", p=128)[:, :, sc * 512:(sc + 1) * 512])
                    for g in range(NG):
                        ps = psp.tile([128, 1024], F32, tag="ps")
                        for dc in range(DC):
                            nc.tensor.matmul(
                                ps[:, 0:512],
                                lhsT=r(w_sb[:, dc, g * 128:(g + 1) * 128]),
                                rhs=r(xc[:, dc, :]),
                                start=(dc == 0), stop=(dc == DC - 1))
                        nc.scalar.activation(
                            o_sb[:, g, sc * 512:(sc + 1) * 512], ps[:, 0:512],
                            AF.Identity, bias=b_sb[:, g:g + 1])
            # v: out[s, hd] = X[s, :] @ W[:, hd]^T-ish (contract din chunks)
            for sc in range(NSC):
                xc = projp.tile([128, DC, 512], F32, tag="xc")
                nc.sync.dma_start(
                    out=xc[:],
                    in_=vt_in.ap().rearrange("(c p) s -> p c s", p=128)[:, :, sc * 512:(sc + 1) * 512])
                for ssub in range(4):  # four 128-row s-slices per 512 chunk
                    ss = sc * 4 + ssub
                    ps = psp.tile([128, 1024], F32, tag="ps")
                    for dc in range(DC):
                        nc.tensor.matmul(
                            ps[:, 0:HG],
                            lhsT=r(xc[:, dc, ssub * 128:(ssub + 1) * 128]),
                            rhs=r(wv_sb[:, dc, :]),
                            start=(dc == 0), stop=(dc == DC - 1))
                    # v_sb[:, ss, h, :] = psum + bv  (broadcast bias row)
                    nc.vector.scalar_tensor_tensor(
                        out=v_sb[:, ss, :, :],
                        in0=ps[:, 0:HG].rearrange("p (h d) -> p h d", h=NHC),
                        scalar=1.0,
                        in1=bv_sb[:].rearrange("p (h d) -> p h d", h=NHC),
                        op0=ALU.mult, op1=ALU.add)

        # ---------------- main loop over heads ----------------
        w2 = ctx.enter_context(tc.tile_pool(name="w2", bufs=2))
        w4 = ctx.enter_context(tc.tile_pool(name="w4", bufs=4))

        for h in range(NHC):
            g = h // 2
            po = 64 * (h % 2)

            # ---- phase A: q-major dist ----
            for qb in range(NQB):
                mq = w2.tile([128, S], BF16, tag="mq")
                nc.gpsimd.dma_start(
                    out=mq[:], in_=maskq_in.ap()[qb * 128:(qb + 1) * 128, :])
                e_blocks = []
                accs = []
                for pb in range(NPB):
                    ps = psp.tile([128, PB], F32, tag="ps")
                    for kc in range(NCH):
                        col = pb * PB + kc * 512
                        nc.tensor.matmul(
                            ps[:, kc * 512:(kc + 1) * 512],
                            lhsT=r(qt_sb[po:po + 64, g, qb * 128:(qb + 1) * 128]),
                            rhs=r(kt_sb[po:po + 64, g, col:col + 512]),
                            start=True, stop=False)
                        nc.tensor.matmul(
                            ps[:, kc * 512:(kc + 1) * 512],
                            lhsT=negI[:],
                            rhs=mq[:, col:col + 512],
                            start=False, stop=True)
                    e = w4.tile([128, PB], F32, tag="e")
                    acc = w4.tile([128, 1], F32, tag="acc")
                    nc.scalar.activation(e[:], ps[:], AF.Exp, scale=0.125,
                                         accum_out=acc[:])
                    e_blocks.append(e)
                    accs.append(acc)
                stot = w4.tile([128, 1], F32, tag="stot")
                if NPB == 2:
                    nc.vector.tensor_add(stot[:], accs[0][:], accs[1][:])
                else:
                    nc.vector.tensor_copy(stot[:], accs[0][:])
                nc.vector.reciprocal(recip_all[:, h, qb:qb + 1], stot[:])
                dist_t = w2.tile([128, S], F32, tag="dist")
                for pb in range(NPB):
                    nc.vector.tensor_scalar_mul(
                        dist_t[:, pb * PB:(pb + 1) * PB],
                        e_blocks[pb][:], recip_all[:, h, qb:qb + 1])
                nc.sync.dma_start(
                    out=dist_out.ap()[h, qb * 128:(qb + 1) * 128, :],
                    in_=dist_t[:])

            # recips -> DRAM row (transposed via strided DMA), then broadcast
            nc.sync.dma_start(
                out=bass.AP(tensor=recip_dram.ap().tensor, offset=h * S,
                            ap=[[1, 128], [128, NQB]]),
                in_=recip_all[:, h, :])

            # ---- phase B: k-major attn ----
            pa = pap.tile([64, S], F32, tag="pa")
            for kb in range(NQB):
                mt = w2.tile([128, S], BF16, tag="mq")
                nc.gpsimd.dma_start(
                    out=mt[:], in_=maskt_in.ap()[kb * 128:(kb + 1) * 128, :])
                for pb in range(NPB):
                    ps = psp.tile([128, PB], F32, tag="ps")
                    for qc in range(NCH):
                        col = pb * PB + qc * 512
                        nc.tensor.matmul(
                            ps[:, qc * 512:(qc + 1) * 512],
                            lhsT=r(kt_sb[po:po + 64, g, kb * 128:(kb + 1) * 128]),
                            rhs=r(qt_sb[po:po + 64, g, col:col + 512]),
                            start=True, stop=False)
                        nc.tensor.matmul(
                            ps[:, qc * 512:(qc + 1) * 512],
                            lhsT=negI[:],
                            rhs=mt[:, col:col + 512],
                            start=False, stop=True)
                    eT = w4.tile([128, PB], F32, tag="e")
                    nc.scalar.activation(eT[:], ps[:], AF.Exp, scale=0.125)
                    for qc in range(NCH):
                        col = pb * PB + qc * 512
                        nc.tensor.matmul(
                            pa[:, col:col + 512],
                            lhsT=r(v_sb[:, kb, h, :]),
                            rhs=r(eT[:, qc * 512:(qc + 1) * 512]),
                            start=(kb == 0), stop=(kb == NQB - 1))

            # normalize attnT rows by 1/rowsum(q) (broadcast across hd lanes)
            rbc = w2.tile([64, S], F32, tag="rbc")
            nc.gpsimd.dma_start(
                out=rbc[:],
                in_=bass.AP(tensor=recip_dram.ap().tensor, offset=h * S,
                            ap=[[0, 64], [1, S]]))
            nc.vector.tensor_mul(atn[:, h, :], pa[:], rbc[:])

        # ---------------- phase C: output projection ----------------
        for qs in range(NQB):
            px = psp.tile([128, 1024], F32, tag="ps")
            for h in range(NHC):
                nc.tensor.matmul(
                    px[:, 0:512],
                    lhsT=r(atn[:, h, qs * 128:(qs + 1) * 128]),
                    rhs=r(wo_sb[:, h, :]),
                    start=(h == 0), stop=(h == NHC - 1))
            xo = w2.tile([128, DM], F32, tag="xo")
            nc.scalar.copy(xo[:], px[:, 0:512])
            nc.sync.dma_start(out=x_out.ap()[qs * 128:(qs + 1) * 128, :],
                              in_=xo[:])

    nc.compile()
    return nc


def make_core_inputs(Q, K, V, mask, Wq, bq, Wk, bk, Wv, bv, Wo, bo,
                     S=2048, DM=512, HD=64, NHC=4):
    """Build the 8 per-core input dicts from full inputs (numpy, host-side)."""
    import ml_dtypes
    HG = NHC * HD
    NG = HG // 128
    negi = (np.eye(128, dtype=np.float32) * NEG_BIG).astype(ml_dtypes.bfloat16)
    in_maps = []
    per_batch = {}
    n_cores = 2 * Q.shape[0]
    for c in range(n_cores):
        b, gq = c // 2, c % 2
        hsl = slice(HG * gq, HG * (gq + 1))
        if b not in per_batch:
            m = np.ascontiguousarray(mask[b, 0]).view(np.uint8)
            mt = np.ascontiguousarray(mask[b, 0].T).view(np.uint8)
            per_batch[b] = (
                np.ascontiguousarray(Q[b].T), np.ascontiguousarray(K[b].T),
                np.ascontiguousarray(V[b].T), m, mt)
        QT, KT, VT, m, mt = per_batch[b]
        in_maps.append({
            "qt_in": QT, "kt_in": KT, "vt_in": VT,
            "maskq_in": m, "maskt_in": mt,
            "wqt_in": np.ascontiguousarray(Wq[hsl].T),
            "wkt_in": np.ascontiguousarray(Wk[hsl].T),
            "wvt_in": np.ascontiguousarray(Wv[hsl].T),
            "wot_in": np.ascontiguousarray(Wo[:, hsl].T),
            "bq_in": np.ascontiguousarray(bq[hsl].reshape(NG, 128).T),
            "bk_in": np.ascontiguousarray(bk[hsl].reshape(NG, 128).T),
            "bv_in": np.ascontiguousarray(bv[hsl].reshape(1, HG)),
            "negi_in": negi,
        })
    return in_maps


def assemble_outputs(results, bo, B=4, S=2048, DM=512, H=8, NHC=4):
    """Gather per-core outputs into full (x, dist)."""
    dist = np.empty((B, H, S, S), np.float32)
    x = np.empty((B, S, DM), np.float32)
    for c in range(8):
        b, gq = c // 2, c % 2
        dist[b, NHC * gq:NHC * (gq + 1)] = results[c]["dist_out"]
    for b in range(B):
        x[b] = results[2 * b]["x_out"] + results[2 * b + 1]["x_out"] + bo[None, :]
    return x, dist


_CACHE = {}


def _get_nc():
    if "nc" not in _CACHE:
        _CACHE["nc"] = build_nc(S=2048, DM=512, num_devices=8)
    return _CACHE["nc"]


def kernel(Q, K, V, mask, Wq, bq, Wk, bk, Wv, bv, Wo, bo):
    from concourse.bass_utils import run_bass_kernel_spmd
    args = dict(Q=np.asarray(Q, np.float32), K=np.asarray(K, np.float32),
                V=np.asarray(V, np.float32), mask=np.asarray(mask),
                Wq=np.asarray(Wq, np.float32), bq=np.asarray(bq, np.float32),
                Wk=np.asarray(Wk, np.float32), bk=np.asarray(bk, np.float32),
                Wv=np.asarray(Wv, np.float32), bv=np.asarray(bv, np.float32),
                Wo=np.asarray(Wo, np.float32), bo=np.asarray(bo, np.float32))
    nc = _get_nc()
    in_maps = make_core_inputs(**args)
    res = run_bass_kernel_spmd(nc, in_maps, core_ids=list(range(8)))
    return assemble_outputs(res.results, args["bo"])


# revision 2
# speedup vs baseline: 1.0770x; 1.0770x over previous
"""Trainium2 Bass kernel for sharded multi-head attention (8 NeuronCores).

Contract: kernel(**inputs) takes the FULL inputs (Q,K,V,mask,W*,b*) and
returns the full (x, dist) outputs, matching reference.reference().

Sharding: 8 cores; core c -> batch b = c//2, head-group g = c%2 (4 heads
per core). Each core computes its 4 heads' dist rows and an x partial;
the host sums the two x partials per batch and adds bo.
"""
"""v3: MHA kernel using DMA-xbar transpose of the bf16 dist matrix.

Per core (4 heads = 2 pairs):
  proj:   qT/kT [dout, s] (bf16), v [s, hd] (bf16, pair-packed cols)
  phase A (q-major): masked scores (PE, split half-identity mask matmuls,
          head-pair row-group interleaving) -> exp+rowsum (ACT) -> recip ->
          dist(bf16) -> DRAM
  phase B: read dist back TRANSPOSED via DMA xbar (bf16) -> attnT = v.T @ distT
          accumulated in a pair-packed [128, S] PSUM tile
  phase C: x = sum_g attnT_pair.T @ Wo_pair -> DRAM (f32)
Host: x = partial0 + partial1 + bo; dist = bf16 -> f32.
"""

from contextlib import ExitStack

import numpy as np

import concourse.bass as bass
import concourse.tile as tile
from concourse import bacc, mybir
from concourse.tile import add_dep_helper

F32 = mybir.dt.float32
BF16 = mybir.dt.bfloat16
AF = mybir.ActivationFunctionType
ALU = mybir.AluOpType

NEG_BIG = -8.0e10


def build_nc(S=2048, DM=512, HD=64, NHC=4, num_devices=8):
    HG = NHC * HD            # 256
    NG = HG // 128           # 2 pairs
    DC = DM // 128           # 4
    NQB = S // 128           # 16
    NSC = S // 512           # 4
    PB = 1024 if S % 1024 == 0 else 512
    NPB = S // PB
    NCH = PB // 512

    nc = bacc.Bacc("TRN2", target_bir_lowering=False, debug=False,
                   num_devices=num_devices)

    qt_in = nc.dram_tensor("qt_in", [DM, S], BF16, kind="ExternalInput")
    kt_in = nc.dram_tensor("kt_in", [DM, S], BF16, kind="ExternalInput")
    vt_in = nc.dram_tensor("vt_in", [DM, S], BF16, kind="ExternalInput")
    maskq_in = nc.dram_tensor("maskq_in", [S, S], BF16, kind="ExternalInput")
    wqt_in = nc.dram_tensor("wqt_in", [DM, HG], BF16, kind="ExternalInput")
    wkt_in = nc.dram_tensor("wkt_in", [DM, HG], BF16, kind="ExternalInput")
    wvt_in = nc.dram_tensor("wvt_in", [DM, HG], BF16, kind="ExternalInput")
    wot_in = nc.dram_tensor("wot_in", [HG, DM], BF16, kind="ExternalInput")
    bq_in = nc.dram_tensor("bq_in", [128, NG], F32, kind="ExternalInput")
    bk_in = nc.dram_tensor("bk_in", [128, NG], F32, kind="ExternalInput")
    bv_in = nc.dram_tensor("bv_in", [1, HG], F32, kind="ExternalInput")
    negi_in = nc.dram_tensor("negi_in", [128, 128], BF16, kind="ExternalInput")

    dist_out = nc.dram_tensor("dist_out", [NHC, S, S], BF16, kind="ExternalOutput")
    x_out = nc.dram_tensor("x_out", [S, DM], F32, kind="ExternalOutput")

    with tile.TileContext(nc) as tc, ExitStack() as ctx:
        persist = ctx.enter_context(tc.tile_pool(name="persist", bufs=1))
        psp = ctx.enter_context(tc.tile_pool(name="psp", bufs=2, space="PSUM"))
        pap = ctx.enter_context(tc.tile_pool(name="pap", bufs=1, space="PSUM"))

        negI = persist.tile([128, 128], BF16)
        nc.sync.dma_start(out=negI[:], in_=negi_in.ap())

        wq_sb = persist.tile([128, DC, HG], BF16)
        wk_sb = persist.tile([128, DC, HG], BF16)
        wv_sb = persist.tile([128, DC, HG], BF16)
        nc.sync.dma_start(out=wq_sb[:], in_=wqt_in.ap().rearrange("(c p) m -> p c m", p=128))
        nc.sync.dma_start(out=wk_sb[:], in_=wkt_in.ap().rearrange("(c p) m -> p c m", p=128))
        nc.sync.dma_start(out=wv_sb[:], in_=wvt_in.ap().rearrange("(c p) m -> p c m", p=128))
        wo_sb = persist.tile([128, NG, DM], BF16)
        nc.sync.dma_start(out=wo_sb[:], in_=wot_in.ap().rearrange("(g p) m -> p g m", p=128))
        bq_sb = persist.tile([128, NG], F32)
        bk_sb = persist.tile([128, NG], F32)
        nc.sync.dma_start(out=bq_sb[:], in_=bq_in.ap())
        nc.sync.dma_start(out=bk_sb[:], in_=bk_in.ap())
        bv_sb = persist.tile([128, HG], F32)
        nc.gpsimd.dma_start(out=bv_sb[:], in_=bass.AP(
            tensor=bv_in.ap().tensor, offset=0, ap=[[0, 128], [1, HG]]))

        qt_sb = persist.tile([128, NG, S], BF16)
        kt_sb = persist.tile([128, NG, S], BF16)
        v_sb = persist.tile([128, NQB, NG, 128], BF16)
        atn = persist.tile([128, NG, S], BF16)

        # ---------------- projections ----------------
        with tc.tile_pool(name="proj", bufs=3) as projp:
            for x_in, w_sb, b_sb, o_sb in (
                (qt_in, wq_sb, bq_sb, qt_sb),
                (kt_in, wk_sb, bk_sb, kt_sb),
            ):
                for sc in range(NSC):
                    xc = projp.tile([128, DC, 512], BF16, tag="xc")
                    nc.sync.dma_start(
                        out=xc[:],
                        in_=x_in.ap().rearrange("(c p) s -> p c s", p=128)[:, :, sc * 512:(sc + 1) * 512])
                    for g in range(NG):
                        ps = psp.tile([128, PB], F32, tag="ps")
                        for dc in range(DC):
                            nc.tensor.matmul(
                                ps[:, 0:512],
                                lhsT=w_sb[:, dc, g * 128:(g + 1) * 128],
                                rhs=xc[:, dc, :],
                                start=(dc == 0), stop=(dc == DC - 1))
                        nc.vector.tensor_scalar(
                            o_sb[:, g, sc * 512:(sc + 1) * 512], ps[:, 0:512],
                            b_sb[:, g:g + 1], None, ALU.add)
            for sc in range(NSC):
                xc = projp.tile([128, DC, 512], BF16, tag="xc")
                nc.sync.dma_start(
                    out=xc[:],
                    in_=vt_in.ap().rearrange("(c p) s -> p c s", p=128)[:, :, sc * 512:(sc + 1) * 512])
                for ssub in range(4):
                    ss = sc * 4 + ssub
                    ps = psp.tile([128, PB], F32, tag="ps")
                    for dc in range(DC):
                        nc.tensor.matmul(
                            ps[:, 0:HG],
                            lhsT=xc[:, dc, ssub * 128:(ssub + 1) * 128],
                            rhs=wv_sb[:, dc, :],
                            start=(dc == 0), stop=(dc == DC - 1))
                    nc.vector.scalar_tensor_tensor(
                        out=v_sb[:, ss, :, :],
                        in0=ps[:, 0:HG].rearrange("p (g d) -> p g d", g=NG),
                        scalar=1.0,
                        in1=bv_sb[:].rearrange("p (g d) -> p g d", g=NG),
                        op0=ALU.mult, op1=ALU.add)

        w3 = ctx.enter_context(tc.tile_pool(name="w3", bufs=3))
        w4 = ctx.enter_context(tc.tile_pool(name="w4", bufs=4))
        w8 = ctx.enter_context(tc.tile_pool(name="w8", bufs=8))

        dist_w_insts = [[] for _ in range(NHC)]

        # ---------------- phase A: q-major dist ----------------
        for qb in range(NQB):
            mq = w3.tile([128, S], BF16, tag="mq")
            nc.sync.dma_start(out=mq[:], in_=maskq_in.ap()[qb * 128:(qb + 1) * 128, :])
            for g in range(NG):
                e_blocks = {0: [], 1: []}
                accs = {0: [], 1: []}
                for pb in range(NPB):
                    ts = [psp.tile([128, PB], F32, tag="ps") for _ in range(2)]
                    # scores: head-pair interleaved (row groups 0-63 / 64-127)
                    for kc in range(NCH):
                        col = pb * PB + kc * 512
                        for h01 in range(2):
                            po = 64 * h01
                            nc.tensor.matmul(
                                ts[h01][:, kc * 512:(kc + 1) * 512],
                                lhsT=qt_sb[po:po + 64, g, qb * 128:(qb + 1) * 128],
                                rhs=kt_sb[po:po + 64, g, col:col + 512],
                                start=True, stop=False)
                    # mask: split half-identity matmuls, alternating row groups
                    for kc in range(NCH):
                        col = pb * PB + kc * 512
                        for h01 in range(2):
                            nc.tensor.matmul(
                                ts[h01][:, kc * 512:(kc + 1) * 512],
                                lhsT=negI[0:64, :], rhs=mq[0:64, col:col + 512],
                                start=False, stop=False)
                            nc.tensor.matmul(
                                ts[h01][:, kc * 512:(kc + 1) * 512],
                                lhsT=negI[64:128, :], rhs=mq[64:128, col:col + 512],
                                start=False, stop=True)
                    for h01 in range(2):
                        e = w4.tile([128, PB], F32, tag="e")
                        acc = w8.tile([128, 1], F32, tag="acc")
                        nc.scalar.activation(e[:], ts[h01][:], AF.Exp, scale=0.125,
                                             accum_out=acc[:])
                        e_blocks[h01].append(e)
                        accs[h01].append(acc)
                for h01 in range(2):
                    h = 2 * g + h01
                    stot = w4.tile([128, 1], F32, tag="stot")
                    if NPB == 2:
                        nc.vector.tensor_add(stot[:], accs[h01][0][:], accs[h01][1][:])
                    else:
                        nc.vector.tensor_copy(stot[:], accs[h01][0][:])
                    recip = w4.tile([128, 1], F32, tag="recip")
                    nc.vector.reciprocal(recip[:], stot[:])
                    dist_t = w3.tile([128, S], BF16, tag="dist")
                    for pb in range(NPB):
                        nc.vector.tensor_scalar_mul(
                            dist_t[:, pb * PB:(pb + 1) * PB],
                            e_blocks[h01][pb][:], recip[:])
                    wi = nc.sync.dma_start(
                        out=dist_out.ap()[h, qb * 128:(qb + 1) * 128, :],
                        in_=dist_t[:])
                    dist_w_insts[h].append(wi)

        # ---------------- phase B: attnT via xbar-transposed dist ----------------
        for g in range(NG):
            pa = pap.tile([128, S], F32, tag="pa")
            for kb in range(NQB):
                for h01 in range(2):
                    h = 2 * g + h01
                    dT = w3.tile([128, S], BF16, tag="dT")
                    ri = nc.sync.dma_start(
                        out=dT[:],
                        in_=dist_out.ap()[h, :, kb * 128:(kb + 1) * 128],
                        transpose=True)
                    for wi in dist_w_insts[h]:
                        add_dep_helper(ri.ins, wi.ins, sync=True,
                                       reason="dist write -> transposed readback")
                    for qc in range(NSC):
                        nc.tensor.matmul(
                            pa[64 * h01:64 * h01 + 64, qc * 512:(qc + 1) * 512],
                            lhsT=v_sb[:, kb, g, 64 * h01:64 * h01 + 64],
                            rhs=dT[:, qc * 512:(qc + 1) * 512],
                            start=(kb == 0), stop=(kb == NQB - 1))
            nc.vector.tensor_copy(atn[:, g, :], pa[:])

        # ---------------- phase C: output projection ----------------
        for qs in range(NQB):
            px = psp.tile([128, PB], F32, tag="ps")
            for g in range(NG):
                nc.tensor.matmul(
                    px[:, 0:512],
                    lhsT=atn[:, g, qs * 128:(qs + 1) * 128],
                    rhs=wo_sb[:, g, :],
                    start=(g == 0), stop=(g == NG - 1))
            xo = w3.tile([128, DM], F32, tag="xo")
            nc.vector.tensor_copy(xo[:], px[:, 0:512])
            nc.sync.dma_start(out=x_out.ap()[qs * 128:(qs + 1) * 128, :],
                              in_=xo[:])

    nc.compile()
    return nc


def make_core_inputs(Q, K, V, mask, Wq, bq, Wk, bk, Wv, bv, Wo, bo,
                     S=2048, DM=512, HD=64, NHC=4):
    import ml_dtypes
    bf16 = ml_dtypes.bfloat16
    HG = NHC * HD
    NG = HG // 128
    negi = (np.eye(128, dtype=np.float32) * NEG_BIG).astype(bf16)
    in_maps = []
    per_batch = {}
    n_cores = 2 * Q.shape[0]
    for c in range(n_cores):
        b, gq = c // 2, c % 2
        hsl = slice(HG * gq, HG * (gq + 1))
        if b not in per_batch:
            per_batch[b] = (
                np.ascontiguousarray(Q[b].T).astype(bf16),
                np.ascontiguousarray(K[b].T).astype(bf16),
                np.ascontiguousarray(V[b].T).astype(bf16),
                mask[b, 0].astype(bf16))
        QT, KT, VT, m = per_batch[b]
        in_maps.append({
            "qt_in": QT, "kt_in": KT, "vt_in": VT, "maskq_in": m,
            "wqt_in": np.ascontiguousarray(Wq[hsl].T).astype(bf16),
            "wkt_in": np.ascontiguousarray(Wk[hsl].T).astype(bf16),
            "wvt_in": np.ascontiguousarray(Wv[hsl].T).astype(bf16),
            "wot_in": np.ascontiguousarray(Wo[:, hsl].T).astype(bf16),
            "bq_in": np.ascontiguousarray(bq[hsl].reshape(NG, 128).T).astype(np.float32),
            "bk_in": np.ascontiguousarray(bk[hsl].reshape(NG, 128).T).astype(np.float32),
            "bv_in": np.ascontiguousarray(bv[hsl].reshape(1, HG)).astype(np.float32),
            "negi_in": negi,
        })
    return in_maps


def assemble_outputs(results, bo, B=4, S=2048, DM=512, H=8, NHC=4):
    dist = np.empty((B, H, S, S), np.float32)
    x = np.empty((B, S, DM), np.float32)
    for c in range(len(results)):
        b, gq = c // 2, c % 2
        dist[b, NHC * gq:NHC * (gq + 1)] = results[c]["dist_out"].astype(np.float32)
    for b in range(B):
        x[b] = results[2 * b]["x_out"] + results[2 * b + 1]["x_out"] + bo[None, :]
    return x, dist


_CACHE = {}


def _get_nc():
    if "nc" not in _CACHE:
        _CACHE["nc"] = build_nc(S=2048, DM=512, num_devices=8)
    return _CACHE["nc"]


def kernel(Q, K, V, mask, Wq, bq, Wk, bk, Wv, bv, Wo, bo):
    from concourse.bass_utils import run_bass_kernel_spmd
    args = dict(Q=np.asarray(Q, np.float32), K=np.asarray(K, np.float32),
                V=np.asarray(V, np.float32), mask=np.asarray(mask),
                Wq=np.asarray(Wq, np.float32), bq=np.asarray(bq, np.float32),
                Wk=np.asarray(Wk, np.float32), bk=np.asarray(bk, np.float32),
                Wv=np.asarray(Wv, np.float32), bv=np.asarray(bv, np.float32),
                Wo=np.asarray(Wo, np.float32), bo=np.asarray(bo, np.float32))
    nc = _get_nc()
    in_maps = make_core_inputs(**args)
    res = run_bass_kernel_spmd(nc, in_maps, core_ids=list(range(8)))
    return assemble_outputs(res.results, args["bo"])
